# revision 1
# baseline (speedup 1.0000x reference)
"""Trainium2 Bass kernel for ContactMapPredictor (v2).

Computes, for B=2, N1=500, N2=800, D=128:
    p1 = h1 @ W1[:D] ; p2 = h2 @ W1[D:]
    hidden[b,n,m,:] = relu(p1[b,n,:] + p2[b,m,:] + b1)
    pred[b,n,m]     = hidden[b,n,m,:] @ W2 + b2
    mask[b,n,m]     = (S1[b,n]!=0) * (S2[b,m]!=0)
    y[b,n,m]        = (contact_map[b,n,m] < 0.5) * mask[b,n,m]
Returns (pred, y, mask) each reshaped [B, N1*N2].

v2 layout: shard N1 across 8 cores, 63 rows/core, each batch-half padded to
64 so a core owns 128 output rows r = b*64 + i (exactly the partition count
and 32-aligned for PSUM groups). Per row, hid = relu(p2 + p1col) is one
fused DVE (or ACT) op over [128, 800] bf16; the W2 reduce uses the shifted
w2g trick so 32 rows accumulate into one PSUM partition group. The mask is
a rank-1 PE matmul m1col^T @ m2row straight into PSUM; y_contact is DVE
elementwise at the tail. mask / y_contact leave the device as uint8 (values
are exactly 0/1) to halve output DMA bytes; host casts back to f32.
"""

import numpy as np
import ml_dtypes

import bass_rust
import concourse.bass as bass
import concourse.tile as tile
import concourse.mybir as mybir
from concourse.bass_utils import run_bass_kernel_spmd

BF16NP = ml_dtypes.bfloat16
F32 = mybir.dt.float32
BF16 = mybir.dt.bfloat16
U8 = mybir.dt.uint8

B, N1, N2, D = 2, 500, 800, 128
NCORES = 8
N1S = 63              # valid rows of N1 per core (8*63 = 504 >= 500)
PAD = 64              # each batch-half padded to 64 rows
ROWS = B * PAD        # 128 output rows per core == partition count
N1P = NCORES * N1S
TH = 0.5
CHUNKS = ((0, 512), (512, 800))  # psum-bank-sized free-dim chunks

# Rows whose hid op runs on ACT instead of DVE (by lane, i.e. r % 32).
# |ACT_LANES| * 4 rows go to ACT; tuned so DVE and ACT finish together.
# HW-measured per-[128,800]-row costs (2026-08-08 slope benches): DVE
# tensor_scalar 269ns (4x mode), ACT activation ~950ns, GPSIMD ~12us (never
# use it for bulk elementwise). 7 lanes (28 rows) is the measured optimum:
# 9 lanes measured +2.0us/rep, GPSIMD offload of 8 rows measured +94us/rep.
ACT_LANES = frozenset({2, 6, 11, 15, 20, 24, 29})


def _split_waits(nc):
    """This container's walrus build accepts at most ONE sync-wait command
    per instruction (any extra raises 'Too many sync wait commands' in
    codegen). Tile routinely attaches 2-3 waits to an instruction. Hoist
    all but the last wait onto same-engine NoOp carriers placed directly
    before the instruction — same-sequencer program order preserves the
    happens-before semantics exactly."""
    for blk in nc.m.functions[0].blocks:
        new = []
        for inst in blk.instructions:
            si = inst.sync_info
            waits = list(si.on_wait) if si and si.on_wait else []
            if len(waits) > 1 and inst.engine != mybir.EngineType.Unassigned:
                for w in waits[:-1]:
                    nop = mybir.InstNoOp(
                        name=nc.get_next_instruction_name(), engine=inst.engine
                    )
                    nop.sync_info = bass_rust.SyncInfo(on_wait=[w], on_update=[])
                    nc.register_instruction(nop)
                    new.append(nop)
                si.on_wait = waits[-1:]
                inst.sync_info = si
            new.append(inst)
        blk.instructions = new


def build_nc(repeat=1):
    """repeat>1 re-runs the main loop body (benchmarking aid: the slope of
    wall time vs repeat isolates device execution time from the per-call
    PJRT/axon dispatch overhead)."""
    nc = bass.Bass("TRN2", target_bir_lowering=False, debug=False)

    h1t_d = nc.declare_dram_parameter("h1t", [B, D, PAD], BF16, isOutput=False)
    h2t_d = nc.declare_dram_parameter("h2t", [B, D, N2], BF16, isOutput=False)
    w1a_d = nc.declare_dram_parameter("w1a", [D, D], BF16, isOutput=False)
    w1b_d = nc.declare_dram_parameter("w1b", [D, D], BF16, isOutput=False)
    w2g_d = nc.declare_dram_parameter("w2g", [D, 63], BF16, isOutput=False)
    b1c_d = nc.declare_dram_parameter("b1c", [D, 1], F32, isOutput=False)
    b2c_d = nc.declare_dram_parameter("b2c", [D, 1], F32, isOutput=False)
    m1g_d = nc.declare_dram_parameter("m1g", [1, ROWS], BF16, isOutput=False)
    m2r_d = nc.declare_dram_parameter("m2r", [B, N2], BF16, isOutput=False)
    cm_d = nc.declare_dram_parameter("cm", [ROWS, N2], F32, isOutput=False)

    pred_d = nc.declare_dram_parameter("pred", [ROWS, N2], F32, isOutput=True)
    mask_d = nc.declare_dram_parameter("mask", [ROWS, N2], U8, isOutput=True)
    ycon_d = nc.declare_dram_parameter("ycon", [ROWS, N2], U8, isOutput=True)

    with tile.TileContext(nc) as tc:
        with (
            tc.tile_pool(name="const", bufs=1) as const,
            tc.tile_pool(name="sb", bufs=1) as sb,
            tc.tile_pool(name="hidp", bufs=5) as hidp,
            tc.tile_pool(name="hidap", bufs=3) as hidap,
            tc.tile_pool(name="pps", bufs=1, space="PSUM") as pps,
            tc.tile_pool(name="predps", bufs=1, space="PSUM") as predps,
        ):
            # ---- critical-path DMAs first ----
            w1b = const.tile([D, D], BF16)
            nc.sync.dma_start(out=w1b[:], in_=w1b_d[:])
            h2sb = [sb.tile([D, N2], BF16, tag=f"h2sb{b}", name=f"h2sb{b}")
                    for b in range(B)]
            nc.sync.dma_start(out=h2sb[0][:, 0:512], in_=h2t_d[0][:, 0:512])
            nc.sync.dma_start(out=h2sb[0][:, 512:N2], in_=h2t_d[0][:, 512:N2])
            w1a = const.tile([D, D], BF16)
            nc.sync.dma_start(out=w1a[:], in_=w1a_d[:])
            h1sb = [sb.tile([D, PAD], BF16, tag=f"h1sb{b}", name=f"h1sb{b}")
                    for b in range(B)]
            nc.sync.dma_start(out=h1sb[0][:], in_=h1t_d[0])
            b1c = const.tile([D, 1], F32)
            nc.sync.dma_start(out=b1c[:], in_=b1c_d[:])
            nc.sync.dma_start(out=h2sb[1][:], in_=h2t_d[1])
            nc.sync.dma_start(out=h1sb[1][:], in_=h1t_d[1])
            w2g = const.tile([D, 63], BF16)
            nc.sync.dma_start(out=w2g[:], in_=w2g_d[:])
            # mask-path inputs on the ACT HWDGE queue, off the SP queue
            m1g = const.tile([1, ROWS], BF16)
            nc.scalar.dma_start(out=m1g[:], in_=m1g_d[:])
            m2r = [const.tile([1, N2], BF16, name=f"m2r{b}") for b in range(B)]
            for b in range(B):
                nc.scalar.dma_start(out=m2r[b][:], in_=m2r_d[b:b + 1, :])
            cmsb = sb.tile([ROWS, N2], F32, tag="cmsb")
            nc.scalar.dma_start(out=cmsb[:], in_=cm_d[:])

            # ---- PE warm-up: HAM releases the clock gate after ~3.4us of
            # sustained activity, so burn that window on zero matmuls while
            # the input DMAs land. ----
            warm0 = const.tile([D, 512], BF16)
            nc.vector.memset(warm0[:], 0.0)
            pred_ps = [
                predps.tile([PAD, N2], F32, tag="pred0", name="pred_ps0"),
                predps.tile([PAD, N2], F32, tag="pred1", name="pred_ps1"),
            ]
            for _ in range(5):
                nc.tensor.matmul(
                    out=pred_ps[0][0:1, 0:512], lhsT=warm0[:, 0:1],
                    rhs=warm0[:], start=True, stop=True, skip_group_check=True,
                )

            # ---- projections: p2T = W1b^T @ h2T ; p1bT = W1a^T @ h1T + b1 ----
            p2sb = []
            p1b = sb.tile([D, ROWS], F32, tag="p1b")
            for b in range(B):
                p2ps = pps.tile([D, N2], F32, tag="p2ps")
                for lo, hi in CHUNKS:
                    nc.tensor.matmul(
                        out=p2ps[:, lo:hi], lhsT=w1b[:], rhs=h2sb[b][:, lo:hi],
                        start=True, stop=True,
                    )
                p2 = sb.tile([D, N2], BF16, tag=f"p2_{b}")
                nc.vector.tensor_copy(out=p2[:], in_=p2ps[:])
                p2sb.append(p2)

                p1ps = pps.tile([D, PAD], F32, tag="p1ps")
                nc.tensor.matmul(
                    out=p1ps[:], lhsT=w1a[:], rhs=h1sb[b][:], start=True, stop=True
                )
                nc.scalar.activation(
                    out=p1b[:, b * PAD:(b + 1) * PAD], in_=p1ps[:],
                    func=mybir.ActivationFunctionType.Identity, bias=b1c[:], scale=1.0,
                )

            b2c = const.tile([D, 1], F32)
            nc.sync.dma_start(out=b2c[:], in_=b2c_d[:])

            # ---- mask = m1^T @ m2 as rank-1 PE matmuls into PSUM (reuses the
            # p2ps space, which is free once the p2 copies land) ----
            mask_ps = pps.tile([ROWS, N2], F32, tag="p2ps", name="mask_ps")
            for b in range(B):
                for lo, hi in CHUNKS:
                    nc.tensor.matmul(
                        out=mask_ps[b * PAD:(b + 1) * PAD, lo:hi],
                        lhsT=m1g[0:1, b * PAD:(b + 1) * PAD],
                        rhs=m2r[b][:, lo:hi],
                        start=True, stop=True, skip_group_check=True,
                    )

            # ---- main loop ----
            predsb = sb.tile([ROWS, N2], F32, tag="predsb")
            for rep in range(repeat):
              for r in range(ROWS):
                  b = r // PAD
                  half = r // 64
                  g = (r // 32) % 2
                  lane = r % 32
                  col = p1b[:, r:r + 1]
                  if lane in ACT_LANES:
                      hid = hidap.tile([D, N2], BF16, tag="hida", name="hida")
                      nc.scalar.activation(
                          out=hid[:], in_=p2sb[b][:],
                          func=mybir.ActivationFunctionType.Relu,
                          bias=col, scale=1.0,
                      )
                  else:
                      hid = hidp.tile([D, N2], BF16, tag="hid", name="hid")
                      nc.vector.tensor_scalar(
                          out=hid[:], in0=p2sb[b][:], scalar1=col, scalar2=0.0,
                          op0=mybir.AluOpType.add, op1=mybir.AluOpType.max,
                      )
                  lhsT = w2g[:, 31 - lane:63 - lane]
                  for lo, hi in CHUNKS:
                      nc.tensor.matmul(
                          out=pred_ps[half][g * 32:(g + 1) * 32, lo:hi],
                          lhsT=lhsT, rhs=hid[:, lo:hi],
                          start=(lane == 0), stop=(lane == 31),
                          skip_group_check=True,
                      )
                  # drain each finished psum half while the other still fills
                  if rep == repeat - 1 and r in (63, ROWS - 1):
                      nc.scalar.activation(
                          out=predsb[64 * half:64 * (half + 1), :],
                          in_=pred_ps[half][:],
                          func=mybir.ActivationFunctionType.Identity,
                          bias=b2c[0:64, :], scale=1.0,
                      )
                      nc.sync.dma_start(
                          out=pred_d[64 * half:64 * (half + 1), :],
                          in_=predsb[64 * half:64 * (half + 1), :],
                      )

            # ---- mask / y_contact tail (DVE slots these into gaps) ----
            cmlt = sb.tile([ROWS, N2], BF16, tag="cmlt")
            nc.vector.tensor_scalar(
                out=cmlt[:], in0=cmsb[:], scalar1=TH, scalar2=None,
                op0=mybir.AluOpType.is_lt,
            )
            maskb = sb.tile([ROWS, N2], BF16, tag="maskb")
            nc.vector.tensor_copy(out=maskb[:], in_=mask_ps[:])
            masku = sb.tile([ROWS, N2], U8, tag="masku")
            nc.vector.tensor_copy(out=masku[:], in_=maskb[:])
            nc.scalar.dma_start(out=mask_d[:], in_=masku[:])
            yconu = sb.tile([ROWS, N2], U8, tag="yconu")
            nc.vector.tensor_tensor(
                out=yconu[:], in0=cmlt[:], in1=maskb[:], op=mybir.AluOpType.mult
            )
            nc.scalar.dma_start(out=ycon_d[:], in_=yconu[:])

    _split_waits(nc)
    return nc


def _marshal(inputs):
    """Full inputs -> list of 8 per-core input maps."""
    S1 = np.asarray(inputs["S_mol1"])                       # [B, N1]
    S2 = np.asarray(inputs["S_mol2"])                       # [B, N2]
    h1 = np.asarray(inputs["h_mol1"], dtype=np.float32)     # [B, N1, D]
    h2 = np.asarray(inputs["h_mol2"], dtype=np.float32)     # [B, N2, D]
    cm = np.asarray(inputs["contact_map"], dtype=np.float32)
    W1 = np.asarray(inputs["W1"], dtype=np.float32)         # [2D, D]
    b1 = np.asarray(inputs["b1"], dtype=np.float32)         # [D]
    W2 = np.asarray(inputs["W2"], dtype=np.float32)         # [D, 1]
    b2 = np.asarray(inputs["b2"], dtype=np.float32)         # [1]

    pad = N1P - N1
    m1p = np.pad((S1 != 0).astype(np.float32), ((0, 0), (0, pad)))
    h1p = np.pad(h1, ((0, 0), (0, pad), (0, 0)))
    cmp_ = np.pad(cm, ((0, 0), (0, pad), (0, 0)))

    h2t = np.ascontiguousarray(h2.transpose(0, 2, 1)).astype(BF16NP)  # [B,D,N2]
    w1a = np.ascontiguousarray(W1[:D]).astype(BF16NP)
    w1b = np.ascontiguousarray(W1[D:]).astype(BF16NP)
    w2g = np.zeros((D, 63), np.float32)
    w2g[:, 31] = W2[:, 0]
    w2g = np.ascontiguousarray(w2g).astype(BF16NP)
    b1c = np.ascontiguousarray(b1.reshape(D, 1))
    b2c = np.full((D, 1), float(b2[0]), np.float32)
    m2r = np.ascontiguousarray((S2 != 0).astype(BF16NP))

    in_maps = []
    for c in range(NCORES):
        sl = slice(c * N1S, (c + 1) * N1S)
        h1s = np.zeros((B, PAD, D), np.float32)
        h1s[:, :N1S, :] = h1p[:, sl, :]
        h1t = np.ascontiguousarray(h1s.transpose(0, 2, 1)).astype(BF16NP)
        m1g = np.zeros((B, PAD), np.float32)
        m1g[:, :N1S] = m1p[:, sl]
        m1g = np.ascontiguousarray(m1g.reshape(1, ROWS)).astype(BF16NP)
        cmc = np.zeros((B, PAD, N2), np.float32)
        cmc[:, :N1S, :] = cmp_[:, sl, :]
        cmc = np.ascontiguousarray(cmc.reshape(ROWS, N2))
        in_maps.append({
            "h1t": h1t, "h2t": h2t, "w1a": w1a, "w1b": w1b, "w2g": w2g,
            "b1c": b1c, "b2c": b2c, "m1g": m1g, "m2r": m2r, "cm": cmc,
        })
    return in_maps


def _gather(results):
    """Per-core outputs -> full-shape tuple (pred, y, mask)."""
    outs = []
    for name in ("pred", "ycon", "mask"):
        per_core = np.stack([results[c][name] for c in range(NCORES)])
        # [NCORES, ROWS, N2] -> [NCORES, B, PAD, N2] -> drop pad row ->
        # [B, NCORES*N1S, N2] -> [B, N1, N2]
        full = per_core.reshape(NCORES, B, PAD, N2)[:, :, :N1S, :]
        full = full.transpose(1, 0, 2, 3).reshape(B, N1P, N2)[:, :N1, :]
        outs.append(np.ascontiguousarray(full.reshape(B, N1 * N2), dtype=np.float32))
    pred, ycon, mask = outs
    return pred, ycon, mask


_NC_CACHE = None


def get_nc():
    global _NC_CACHE
    if _NC_CACHE is None:
        _NC_CACHE = build_nc()
    return _NC_CACHE


def kernel(**inputs):
    nc = get_nc()
    in_maps = _marshal(inputs)
    res = run_bass_kernel_spmd(nc, in_maps, core_ids=list(range(NCORES)))
    return _gather(res.results)

